# revision 27
# baseline (speedup 1.0000x reference)
"""Distributed causal attention w/ RoPE for TRN2 (8 NeuronCores).

Sharding: 2-way batch x 4-way head-group. Core r: batch b=r//4, quad
q=r%4, heads 4q..4q+3 as two pairs. Per core:
  - QKV projection of its batch only. q/k in transposed pair layout
    [128=2*64 dims, T] with fused RoPE (pair-swap via strided DMA, mul/add
    on DVE). v projected directly in natural [tk, d] layout (x tile as
    stationary operand) -- no transposes; a ones column is appended per
    head so the AV matmul also produces the softmax denominator (M=65).
  - Scores computed transposed [tk, tq] in wide groups: 2 tk-tiles x 2
    heads land in one 4-bank PSUM tile [128, 2048]; the two heads' score
    matmuls use disjoint PE row groups (K=64 at base partitions 0/64) so
    they run concurrently. One wide exp ACTIVATE per group with the 1/8
    softmax scale folded into the activation's scale field; causal edge
    handled by additive -30000 triangle on PSUM before exp.
  - Normalize: reciprocal of the den row (partition 64) on DVE,
    broadcast across 64 partitions with a K=1 ones matmul, then one
    scalar_tensor_tensor multiply.
  - Output projection partials (K=256) -> chunked ReduceScatter within
    the 4-core batch group, overlapped with the next query group.
Emission interleaves attention groups with the next chunk's QKV pieces
and the previous group's output-projection pieces to keep PE/ACT/DVE all
fed.
"""

import numpy as np

import concourse.bass as bass
import concourse.bacc as bacc
import concourse.mybir as mybir
from concourse import tile
from concourse.bass_utils import run_bass_kernel_spmd

B, T, C, H, D = 2, 2048, 1024, 16, 64
NCORE = 8
TCH = 512                 # token chunk (query group)
NTC = T // TCH            # 4
NBLK = T // 128           # 16 tk blocks
ROPE_BASE = 10000.0
NEG = -30000.0
F32 = mybir.dt.float32
F16 = mybir.dt.float16
MUL = mybir.AluOpType.mult


def _rope_tables():
    d = np.arange(D)
    j = d // 2
    theta = ROPE_BASE ** (-(2.0 * j) / D)
    t = np.arange(T, dtype=np.float64)
    ang = t[None, :] * theta[:, None]
    cos = np.cos(ang)
    sin = np.sin(ang)
    sgn = np.where(d % 2 == 0, -1.0, 1.0)[:, None]
    c1 = np.concatenate([cos, cos], axis=0).astype(np.float16)
    s1 = np.concatenate([sgn * sin, sgn * sin], axis=0).astype(np.float16)
    return c1, s1


def _tri():
    tk = np.arange(128)[:, None]
    jj = np.arange(128)[None, :]
    return np.where(jj >= tk, 0.0, NEG).astype(np.float32)


def build(debug=False):
    nc = bacc.Bacc(num_devices=NCORE)
    x_t = nc.declare_dram_parameter("x_t", [C, T], F16, isOutput=False)
    w_qk = nc.declare_dram_parameter("w_qk", [C, 512], F16, isOutput=False)
    w_v = nc.declare_dram_parameter("w_v", [C, 256], F16, isOutput=False)
    w_p = nc.declare_dram_parameter("w_p", [256, C], F16, isOutput=False)
    out_ext = nc.declare_dram_parameter("out", [NTC, 256, TCH], F16,
                                        isOutput=True)
    dbg = {}
    if debug:
        dbg["vaug0"] = nc.declare_dram_parameter("dbg_vaug0", [128, 4, 65],
                                                 F16, isOutput=True)
        dbg["rq0"] = nc.declare_dram_parameter("dbg_rq0", [128, T], F16,
                                               isOutput=True)
        dbg["rk0"] = nc.declare_dram_parameter("dbg_rk0", [128, T], F16,
                                               isOutput=True)
        dbg["e0"] = nc.declare_dram_parameter("dbg_e0", [128, 2048], F16,
                                              isOutput=True)
        dbg["yraw"] = nc.declare_dram_parameter("dbg_yraw", [65, TCH], F32,
                                                isOutput=True)
        dbg["rec"] = nc.declare_dram_parameter("dbg_rec", [1, TCH], F32,
                                               isOutput=True)
        dbg["rbsb"] = nc.declare_dram_parameter("dbg_rbsb", [64, TCH], F16,
                                                isOutput=True)
        dbg["ysb"] = nc.declare_dram_parameter("dbg_ysb", [128, TCH], F16,
                                               isOutput=True)

    c1_np, s1_np = _rope_tables()
    cost_c = nc.inline_tensor(c1_np, name="cost")
    sint_c = nc.inline_tensor(s1_np, name="sint")
    tri_c = nc.inline_tensor(_tri(), name="tri")
    ones_c = nc.inline_tensor(np.ones((128, 64), np.float16), name="ones1")

    cc_in = [nc.dram_tensor(f"cc_in{g}", [C, TCH], F16) for g in range(NTC)]
    cc_out = [nc.dram_tensor(f"cc_out{g}", [256, TCH], F16)
              for g in range(NTC)]
    groups = [[0, 1, 2, 3], [4, 5, 6, 7]]

    with tile.TileContext(nc) as tc:
        with (
            tc.tile_pool(name="spsum", bufs=1, space="PSUM") as spool,
            tc.tile_pool(name="ypsum", bufs=1, space="PSUM") as ypsum,
            tc.tile_pool(name="mm", bufs=2, space="PSUM") as mmpool,
            tc.tile_pool(name="const", bufs=1) as cpool,
            tc.tile_pool(name="xt", bufs=16) as xpool,
            tc.tile_pool(name="tmp", bufs=6) as tpool,
            tc.tile_pool(name="exp", bufs=3) as epool,
            tc.tile_pool(name="ysb", bufs=4) as ypool,
            tc.tile_pool(name="osb", bufs=4) as opool,
            tc.tile_pool(name="small", bufs=4) as smpool,
        ):
            # ---- persistent SBUF loads --------------------------------
            wqk_sb = []
            for c8 in range(8):
                w = cpool.tile([128, 512], F16, tag=f"wqk{c8}", name=f"wqk{c8}")
                nc.sync.dma_start(out=w[:, :],
                                  in_=w_qk[c8 * 128:(c8 + 1) * 128, :])
                wqk_sb.append(w)
            wv_sb = []
            for c8 in range(8):
                w = cpool.tile([128, 256], F16, tag=f"wv{c8}", name=f"wv{c8}")
                nc.sync.dma_start(out=w[:, :],
                                  in_=w_v[c8 * 128:(c8 + 1) * 128, :])
                wv_sb.append(w)
            wp_sb = []
            for k2 in range(2):
                w = cpool.tile([128, C], F16, tag=f"wp{k2}", name=f"wp{k2}")
                nc.sync.dma_start(out=w[:, :],
                                  in_=w_p[k2 * 128:(k2 + 1) * 128, :])
                wp_sb.append(w)
            cost_sb = cpool.tile([128, T], F16, tag="cost", name="cost_sb")
            nc.sync.dma_start(out=cost_sb[:, :], in_=cost_c[:, :])
            sint_sb = cpool.tile([128, T], F16, tag="sint", name="sint_sb")
            nc.sync.dma_start(out=sint_sb[:, :], in_=sint_c[:, :])
            tri_sb = cpool.tile([128, 128], F32, tag="tri", name="tri_sb")
            nc.sync.dma_start(out=tri_sb[:, :], in_=tri_c[:, :])
            ones_sb = cpool.tile([128, 64], F16, tag="ones", name="ones_sb")
            nc.sync.dma_start(out=ones_sb[:, :], in_=ones_c[:, :])

            rope_q = [cpool.tile([128, T], F16, tag=f"rq{p}", name=f"rq{p}")
                      for p in range(2)]
            rope_k = [cpool.tile([128, T], F16, tag=f"rk{p}", name=f"rk{p}")
                      for p in range(2)]
            vaug = cpool.tile([128, NBLK, 4, 65], F16, tag="vaug",
                              name="vaug")
            nc.vector.memset(vaug[:, :, :, 64], 1.0)

            # ---- emission helpers -------------------------------------
            def qkv_pieces(c):
                """Return list of thunks, each emitting one 8-MM piece."""
                t0 = c * TCH
                xts = []

                def load_x():
                    for c8 in range(8):
                        xt = xpool.tile([128, TCH], F16, tag="xt", name="xt")
                        nc.sync.dma_start(
                            out=xt[:, :],
                            in_=x_t[c8 * 128:(c8 + 1) * 128, t0:t0 + TCH])
                        xts.append(xt)

                def qk_piece(p, base, rope_dst):
                    ps = mmpool.tile([128, TCH], F32, tag="mm", name="qkps")
                    for c8 in range(8):
                        nc.tensor.matmul(
                            ps[:, :], wqk_sb[c8][:, base:base + 128],
                            xts[c8][:, :], start=(c8 == 0), stop=(c8 == 7))
                    m_sb = tpool.tile([128, TCH], F16, tag="msb", name="msb")
                    nc.vector.tensor_copy(m_sb[:, :], ps[:, :])
                    ms_sb = tpool.tile([128, TCH], F16, tag="mssb",
                                       name="mssb")
                    nc.sync.dma_start(out=ms_sb[0::2, :], in_=m_sb[1::2, :])
                    nc.sync.dma_start(out=ms_sb[1::2, :], in_=m_sb[0::2, :])
                    t1 = tpool.tile([128, TCH], F16, tag="t1", name="t1")
                    nc.gpsimd.tensor_mul(t1[:, :], m_sb[:, :],
                                         cost_sb[:, t0:t0 + TCH])
                    t2 = tpool.tile([128, TCH], F16, tag="t2", name="t2")
                    nc.gpsimd.tensor_mul(t2[:, :], ms_sb[:, :],
                                         sint_sb[:, t0:t0 + TCH])
                    nc.gpsimd.tensor_add(rope_dst[:, t0:t0 + TCH],
                                         t1[:, :], t2[:, :])

                def v_piece(j):  # two tk blocks per piece
                    vps = mmpool.tile([128, 2, 4, 64], F32, tag="mm",
                                      name="vps")
                    for i in range(2):
                        blk = 2 * j + i
                        for c8 in range(8):
                            nc.tensor.matmul(
                                vps[:, i, :, :],
                                xts[c8][:, 128 * blk:128 * blk + 128],
                                wv_sb[c8][:, :],
                                start=(c8 == 0), stop=(c8 == 7))
                    tb = 4 * c + 2 * j
                    nc.scalar.copy(vaug[:, tb:tb + 2, :, 0:64],
                                   vps[:, :, :, :])

                pieces = [load_x]
                for p in range(2):
                    pieces.append(lambda p=p: qk_piece(p, 128 * p, rope_q[p]))
                    pieces.append(
                        lambda p=p: qk_piece(p, 256 + 128 * p, rope_k[p]))
                pieces.append(lambda: v_piece(0))
                pieces.append(lambda: v_piece(1))
                return pieces

            EXP = mybir.ActivationFunctionType.Exp

            def att_groups(g):
                """Return list of thunks: score-group bundles + normalize,
                pair-major."""
                t0 = g * TCH
                bundles = []
                for p in range(2):
                    yps = [None, None]

                    def nd_group(grp, p=p, yps=yps):
                        s4 = spool.tile([128, 4 * TCH], F32, tag="s",
                                        name="s4")
                        for tt in range(2):
                            Tt = 2 * grp + tt
                            for h in range(2):
                                off = (512 * tt) + (1024 * h)
                                nc.tensor.matmul(
                                    s4[:, off:off + 512],
                                    rope_k[p][64 * h:64 * h + 64,
                                              128 * Tt:128 * Tt + 128],
                                    rope_q[p][64 * h:64 * h + 64,
                                              t0:t0 + TCH],
                                    start=True, stop=True)
                        e = epool.tile([128, 4 * TCH], F16, tag="e", name="e")
                        nc.scalar.activation(e[:, :], s4[:, :], EXP,
                                             scale=0.125)

                        def av(grp=grp, p=p, yps=yps, e=e):
                            first = (grp == 0)
                            for h in range(2):
                                for tt in range(2):
                                    Tt = 2 * grp + tt
                                    off = (512 * tt) + (1024 * h)
                                    nc.tensor.matmul(
                                        yps[h][:, :],
                                        vaug[:, Tt, 2 * p + h, :],
                                        e[:, off:off + 512],
                                        start=(first and tt == 0),
                                        stop=False)
                        return av

                    def d1_group(p=p, yps=yps):
                        # layout: h0-r0 @0:512, h0-r1 @512:896 (dead 896:1024)
                        #         h1-r0 @1024:1536, h1-r1 @1536:1920 (dead
                        #         1920:2048) -- every MM within one bank
                        s4 = spool.tile([128, 4 * TCH], F32, tag="s",
                                        name="s4d1")
                        nc.vector.memset(s4[:, 896:1024], NEG)
                        nc.vector.memset(s4[:, 1920:2048], NEG)
                        for h in range(2):  # r0 pair, row-group concurrent
                            nc.tensor.matmul(
                                s4[:, 1024 * h:1024 * h + 512],
                                rope_k[p][64 * h:64 * h + 64,
                                          128 * 4 * g:128 * 4 * g + 128],
                                rope_q[p][64 * h:64 * h + 64, t0:t0 + TCH],
                                start=True, stop=True)
                        for h in range(2):  # r1 pair
                            nc.tensor.matmul(
                                s4[:, 1024 * h + 512:1024 * h + 896],
                                rope_k[p][64 * h:64 * h + 64,
                                          128 * (4 * g + 1):
                                          128 * (4 * g + 1) + 128],
                                rope_q[p][64 * h:64 * h + 64,
                                          t0 + 128:t0 + TCH],
                                start=True, stop=True)
                        for h in range(2):
                            hb = 1024 * h
                            nc.vector.tensor_add(s4[:, hb:hb + 128],
                                                 s4[:, hb:hb + 128],
                                                 tri_sb[:, :])
                            nc.vector.tensor_add(s4[:, hb + 512:hb + 640],
                                                 s4[:, hb + 512:hb + 640],
                                                 tri_sb[:, :])
                        e = epool.tile([128, 4 * TCH], F16, tag="e",
                                       name="ed1")
                        nc.scalar.activation(e[:, :], s4[:, :], EXP,
                                             scale=0.125)
                        if debug and g == 0 and p == 0:
                            nc.sync.dma_start(out=dbg["e0"][:, :], in_=e[:, :])

                        def av(p=p, yps=yps, e=e):
                            first = (g == 0)
                            for h in range(2):
                                hb = 1024 * h
                                nc.tensor.matmul(
                                    yps[h][:, :],
                                    vaug[:, 4 * g, 2 * p + h, :],
                                    e[:, hb:hb + 512],
                                    start=first, stop=False)
                                nc.tensor.matmul(
                                    yps[h][:, 128:TCH],
                                    vaug[:, 4 * g + 1, 2 * p + h, :],
                                    e[:, hb + 512:hb + 896],
                                    start=False, stop=False)
                        return av

                    def d2_group(p=p, yps=yps):
                        # layout: h0-r2 @0:256, h0-r3 @256:384 (dead 384:512)
                        #         h1-r2 @512:768, h1-r3 @768:896
                        s4 = spool.tile([128, 4 * TCH], F32, tag="s",
                                        name="s4d2")
                        nc.vector.memset(s4[:, 384:512], NEG)
                        for h in range(2):  # r2 pair
                            nc.tensor.matmul(
                                s4[:, 512 * h:512 * h + 256],
                                rope_k[p][64 * h:64 * h + 64,
                                          128 * (4 * g + 2):
                                          128 * (4 * g + 2) + 128],
                                rope_q[p][64 * h:64 * h + 64,
                                          t0 + 256:t0 + TCH],
                                start=True, stop=True)
                        for h in range(2):  # r3 pair
                            nc.tensor.matmul(
                                s4[:, 512 * h + 256:512 * h + 384],
                                rope_k[p][64 * h:64 * h + 64,
                                          128 * (4 * g + 3):
                                          128 * (4 * g + 3) + 128],
                                rope_q[p][64 * h:64 * h + 64,
                                          t0 + 384:t0 + TCH],
                                start=True, stop=True)
                        for h in range(2):
                            hb = 512 * h
                            nc.vector.tensor_add(s4[:, hb:hb + 128],
                                                 s4[:, hb:hb + 128],
                                                 tri_sb[:, :])
                            nc.vector.tensor_add(s4[:, hb + 256:hb + 384],
                                                 s4[:, hb + 256:hb + 384],
                                                 tri_sb[:, :])
                        e = epool.tile([128, 4 * TCH], F16, tag="e",
                                       name="ed2")
                        nc.scalar.activation(e[:, 0:896], s4[:, 0:896], EXP,
                                             scale=0.125)

                        def av(p=p, yps=yps, e=e):
                            for h in range(2):
                                hb = 512 * h
                                nc.tensor.matmul(
                                    yps[h][:, 256:TCH],
                                    vaug[:, 4 * g + 2, 2 * p + h, :],
                                    e[:, hb:hb + 256],
                                    start=False, stop=False)
                                nc.tensor.matmul(
                                    yps[h][:, 384:TCH],
                                    vaug[:, 4 * g + 3, 2 * p + h, :],
                                    e[:, hb + 256:hb + 384],
                                    start=False, stop=True)
                        return av

                    def normalize(p=p, yps=yps):
                        y_sb = ypool.tile([128, TCH], F16, tag=f"ysb{p}",
                                          name=f"ysb{p}")
                        if debug and g == 0 and p == 0:
                            y32 = smpool.tile([65, TCH], F32, tag="y32",
                                              name="y32")
                            nc.vector.tensor_copy(y32[:, :], yps[0][:, :])
                            nc.sync.dma_start(out=dbg["yraw"][:, :],
                                              in_=y32[:, :])
                        for h in range(2):
                            # den (partition 64, scaled 1/16 into f16) ->
                            # broadcast to partitions 0..63 via K=1 matmul
                            # at row offset 64 -> reciprocal at base
                            # partition 0 (custom DVE ops ignore the input
                            # AP base partition) -> y * (1/16) * 16/den.
                            den16 = smpool.tile([65, TCH], F16, tag="den16",
                                                name="den16")
                            nc.scalar.activation(
                                den16[64:65, :], yps[h][64:65, :],
                                mybir.ActivationFunctionType.Copy,
                                scale=0.0625)
                            dbc = mmpool.tile([128, TCH], F32, tag="mm",
                                              name="dbc")
                            nc.tensor.matmul(dbc[0:64, :],
                                             ones_sb[64:65, :],
                                             den16[64:65, :], start=True,
                                             stop=True)
                            rec = smpool.tile([64, TCH], F32, tag="rec",
                                              name="rec")
                            scr = smpool.tile([64, TCH], F32, tag="scr",
                                              name="scr")
                            nc.vector.reciprocal_approx_accurate(
                                rec[:, :], dbc[0:64, :], scr[:, :])
                            nc.vector.scalar_tensor_tensor(
                                y_sb[64 * h:64 * h + 64, :],
                                yps[h][0:64, :], 0.0625, rec[:, :],
                                op0=MUL, op1=MUL)
                            if debug and g == 0 and p == 0 and h == 0:
                                nc.sync.dma_start(out=dbg["rec"][:, :],
                                                  in_=rec[0:1, :])
                                nc.sync.dma_start(out=dbg["rbsb"][0:1, :],
                                                  in_=den16[64:65, :])
                        if debug and g == 0 and p == 0:
                            nc.sync.dma_start(out=dbg["ysb"][:, :],
                                              in_=y_sb[:, :])
                        y_tiles[p] = y_sb

                    # group order: non-diag pairs, then diag1, diag2.
                    # Software-pipelined: bundle i emits scores+exp of
                    # group i, then the AV matmuls of group i-1 (whose exp
                    # is complete by then), so the PE never waits on ACT.
                    ng = 2 * g
                    state = {"av": None}

                    grp_fns = [lambda grp=grp, f=nd_group: f(grp)
                               for grp in range(ng)]
                    grp_fns.append(d1_group)
                    grp_fns.append(d2_group)

                    def piped(spec, first=False, yps=yps, state=state):
                        def run():
                            if first:
                                for h in range(2):
                                    yps[h] = ypsum.tile(
                                        [65, TCH], F32, tag=f"y{h}",
                                        name=f"y{h}")
                            av = spec()
                            prev = state["av"]
                            state["av"] = av
                            if prev:
                                prev()
                        return run

                    def tail(fn=normalize, state=state):
                        def run():
                            state["av"]()
                            state["av"] = None
                            fn()
                        return run

                    bundles.append(piped(grp_fns[0], first=True))
                    bundles.extend(piped(f) for f in grp_fns[1:])
                    bundles.append(tail())
                return bundles

            y_tiles = [None, None]

            def op_pieces(g):
                """8 m-tile pieces + RS + out DMA as thunks. Captures the
                y tiles at call time (before the next group overwrites
                y_tiles)."""
                y0, y1 = y_tiles[0], y_tiles[1]
                pieces = []

                def m_piece(m8, y0=y0, y1=y1):
                    op_ps = mmpool.tile([128, TCH], F32, tag="mm",
                                        name="opps")
                    nc.tensor.matmul(op_ps[:, :],
                                     wp_sb[0][:, 128 * m8:128 * m8 + 128],
                                     y0[:, :], start=True, stop=False)
                    nc.tensor.matmul(op_ps[:, :],
                                     wp_sb[1][:, 128 * m8:128 * m8 + 128],
                                     y1[:, :], start=False, stop=True)
                    o_sb = opool.tile([128, TCH], F16, tag="osb", name="osb")
                    if m8 % 2 == 0:
                        nc.vector.tensor_copy(o_sb[:, :], op_ps[:, :])
                    else:
                        nc.scalar.copy(o_sb[:, :], op_ps[:, :])
                    nc.sync.dma_start(
                        out=cc_in[g][128 * m8:128 * m8 + 128, :],
                        in_=o_sb[:, :])

                for m8 in range(8):
                    pieces.append(lambda m8=m8: m_piece(m8))

                def rs():
                    nc.gpsimd.collective_compute(
                        "ReduceScatter", mybir.AluOpType.add,
                        replica_groups=groups,
                        ins=[cc_in[g].ap().opt()],
                        outs=[cc_out[g].ap().opt()])
                    nc.sync.dma_start(out=out_ext[g, :, :],
                                      in_=cc_out[g][:, :])

                pieces.append(rs)
                return pieces

            def interleave(bundles, fillers):
                nb, nf = len(bundles), len(fillers)
                fi = 0
                for i, bnd in enumerate(bundles):
                    bnd()
                    # emit fillers proportionally after each bundle
                    want = (i + 1) * nf // nb
                    while fi < want:
                        fillers[fi]()
                        fi += 1
                while fi < nf:
                    fillers[fi]()
                    fi += 1

            # ---- main emission ----------------------------------------
            for f in qkv_pieces(0):
                f()
            if debug:
                nc.sync.dma_start(out=dbg["vaug0"][:, :, :],
                                  in_=vaug[:, 0, :, :])
            pending_op = []
            for g in range(NTC):
                fillers = list(pending_op)
                if g + 1 < NTC:
                    fillers += qkv_pieces(g + 1)
                interleave(att_groups(g), fillers)
                pending_op = op_pieces(g)
            for f in pending_op:
                f()
            if debug:
                nc.sync.dma_start(out=dbg["rq0"][:, :], in_=rope_q[0][:, :])
                nc.sync.dma_start(out=dbg["rk0"][:, :], in_=rope_k[0][:, :])

    if not nc.is_finalized():
        nc.finalize()
    return nc


_NC_CACHE = None


def _get_nc():
    global _NC_CACHE
    if _NC_CACHE is None:
        _NC_CACHE = build()
    return _NC_CACHE


def make_in_maps(x, w_qkv, w_proj):
    x = np.asarray(x, np.float32)
    w_qkv = np.asarray(w_qkv, np.float32)
    w_proj = np.asarray(w_proj, np.float32)
    x_tb = [np.ascontiguousarray(x[b].T).astype(np.float16)
            for b in range(B)]
    maps = []
    for r in range(NCORE):
        b, q = divmod(r, 4)
        heads = [4 * q + i for i in range(4)]
        qcols, kcols = [], []
        for p in range(2):
            hs = heads[2 * p:2 * p + 2]
            rows = [h * 64 + d for h in hs for d in range(D)]
            qcols.append(w_qkv[rows, :].T)
            kcols.append(w_qkv[[C + i for i in rows], :].T)
        w_qk = np.ascontiguousarray(
            np.concatenate(qcols + kcols, axis=1)).astype(np.float16)
        vrows = [2 * C + h * 64 + d for h in heads for d in range(D)]
        w_v = np.ascontiguousarray(w_qkv[vrows, :].T).astype(np.float16)
        mydims = [h * 64 + d for h in heads for d in range(D)]
        w_p = np.ascontiguousarray(w_proj[:, mydims].T).astype(np.float16)
        maps.append({"x_t": x_tb[b], "w_qk": w_qk, "w_v": w_v, "w_p": w_p})
    return maps


def assemble(results):
    outT = np.zeros((B, C, T), np.float32)
    for r in range(NCORE):
        b, q = divmod(r, 4)
        o = results[r]["out"].astype(np.float32)
        for g in range(NTC):
            outT[b, 256 * q:256 * (q + 1), g * TCH:(g + 1) * TCH] = o[g]
    return np.ascontiguousarray(outT.transpose(0, 2, 1))


def run(x, w_qkv, w_proj, trace=False):
    nc = _get_nc()
    in_maps = make_in_maps(x, w_qkv, w_proj)
    res = run_bass_kernel_spmd(nc, in_maps, list(range(NCORE)), trace=trace)
    return assemble(res.results), res


def kernel(x, w_qkv, w_proj):
    out, _ = run(x, w_qkv, w_proj, trace=False)
    return out


# revision 36
# speedup vs baseline: 1.2641x; 1.2641x over previous
"""Distributed causal attention w/ RoPE for TRN2 (8 NeuronCores).

Sharding: 2-way batch x 4-way head-group. Core r: batch b=r//4, quad
q=r%4, heads 4q..4q+3 as two pairs. Per core:
  - QKV projection of its batch only. q/k in transposed pair layout
    [128=2*64 dims, T] with fused RoPE (pair-swap via strided DMA, mul/add
    on GpSimd). v projected directly in natural [tk, d] layout (x tile as
    stationary operand) -- no transposes; a ones column is appended per
    head so the AV matmul also produces the softmax denominator (M=65).
  - Scores computed transposed [tk, tq]: one tk-tile x 2 heads per
    2-bank PSUM tile [128, 1024], double-buffered so score matmuls for
    group i+1 overlap the exp of group i. The two heads' score matmuls
    use disjoint PE row groups (K=64 at base partitions 0/64) and run
    concurrently. One exp ACTIVATE per group with the 1/8 softmax scale
    folded into the activation scale; causal triangle applied as an
    accumulated identity x (-30000 mask) matmul on the PE.
  - Software pipelining: group i's AV matmuls are emitted after group
    i+1's scores+exp, so the PE never waits on ACT.
  - Normalize: den row (partition 64) scaled-copy to f16, broadcast to
    partitions 0..63 with a K=1 ones matmul at row offset 64, reciprocal
    at base partition 0 (custom DVE ops ignore AP base partition), one
    scalar_tensor_tensor multiply.
  - Output projection partials (K=256) -> per-half ReduceScatters within
    the 4-core batch group, overlapped with the next query group.
"""

import numpy as np

import concourse.bass as bass
import concourse.bacc as bacc
import concourse.mybir as mybir
from concourse import tile
from concourse.bass_utils import run_bass_kernel_spmd

B, T, C, H, D = 2, 2048, 1024, 16, 64
NCORE = 8
TCH = 512                 # token chunk (query group)
NTC = T // TCH            # 4
NBLK = T // 128           # 16 tk blocks
ROPE_BASE = 10000.0
NEG = -30000.0
F32 = mybir.dt.float32
F16 = mybir.dt.float16
MUL = mybir.AluOpType.mult
EXP = mybir.ActivationFunctionType.Exp
CPY = mybir.ActivationFunctionType.Copy


def _rope_tables():
    d = np.arange(D)
    j = d // 2
    theta = ROPE_BASE ** (-(2.0 * j) / D)
    t = np.arange(T, dtype=np.float64)
    ang = t[None, :] * theta[:, None]
    cos = np.cos(ang)
    sin = np.sin(ang)
    sgn = np.where(d % 2 == 0, -1.0, 1.0)[:, None]
    c1 = np.concatenate([cos, cos], axis=0).astype(np.float16)
    s1 = np.concatenate([sgn * sin, sgn * sin], axis=0).astype(np.float16)
    return c1, s1


def _tri():
    tk = np.arange(128)[:, None]
    jj = np.arange(128)[None, :]
    return np.where(jj >= tk, 0.0, NEG).astype(np.float16)


def build(debug=False):
    nc = bacc.Bacc(num_devices=NCORE)
    x_t = nc.declare_dram_parameter("x_t", [C, T], F16, isOutput=False)
    w_qk = nc.declare_dram_parameter("w_qk", [C, 512], F16, isOutput=False)
    w_v = nc.declare_dram_parameter("w_v", [C, 256], F16, isOutput=False)
    w_p = nc.declare_dram_parameter("w_p", [256, C], F16, isOutput=False)
    out_ext = nc.declare_dram_parameter("out", [NTC, 2, 128, TCH], F16,
                                        isOutput=True)
    dbg = {}
    if debug:
        dbg["e0"] = nc.declare_dram_parameter("dbg_e0", [128, 1024], F16,
                                              isOutput=True)
        dbg["s0"] = nc.declare_dram_parameter("dbg_s0", [128, 1024], F32,
                                              isOutput=True)
        dbg["yraw"] = nc.declare_dram_parameter("dbg_yraw", [65, TCH], F32,
                                                isOutput=True)
        dbg["rec"] = nc.declare_dram_parameter("dbg_rec", [1, TCH], F32,
                                               isOutput=True)
        dbg["ysb"] = nc.declare_dram_parameter("dbg_ysb", [128, TCH], F16,
                                               isOutput=True)

    c1_np, s1_np = _rope_tables()
    cost_c = nc.inline_tensor(c1_np, name="cost")
    sint_c = nc.inline_tensor(s1_np, name="sint")
    tri_c = nc.inline_tensor(_tri(), name="tri")
    ident_c = nc.inline_tensor(np.eye(128, dtype=np.float16), name="ident")
    ones_c = nc.inline_tensor(np.ones((128, 64), np.float16), name="ones1")

    cc_in = [nc.dram_tensor(f"cc_in{g}", [C, TCH], F16) for g in range(NTC)]
    cc_out = [[nc.dram_tensor(f"cc_out{g}_{hh}", [128, TCH], F16)
               for hh in range(2)] for g in range(NTC)]
    groups = [[0, 1, 2, 3], [4, 5, 6, 7]]

    with tile.TileContext(nc) as tc:
        with (
            tc.tile_pool(name="spsum", bufs=2, space="PSUM") as spool,
            tc.tile_pool(name="ypsum", bufs=1, space="PSUM") as ypsum,
            tc.tile_pool(name="mm", bufs=2, space="PSUM") as mmpool,
            tc.tile_pool(name="const", bufs=1) as cpool,
            tc.tile_pool(name="xt", bufs=16) as xpool,
            tc.tile_pool(name="tmp", bufs=6) as tpool,
            tc.tile_pool(name="exp", bufs=4) as epool,
            tc.tile_pool(name="ysb", bufs=4) as ypool,
            tc.tile_pool(name="osb", bufs=4) as opool,
            tc.tile_pool(name="small", bufs=4) as smpool,
        ):
            # ---- persistent SBUF tiles --------------------------------
            # (x chunk-0 + w_qk loads go first on the sync queue; wv and
            # the rope tables on the vector queue; wp/tri/ident/ones on
            # the scalar queue -- parallel DMA rings, and nothing the
            # first matmuls need is queued behind cold constants.)
            wqk_sb = []
            for c8 in range(8):
                w = cpool.tile([128, 512], F16, tag=f"wqk{c8}", name=f"wqk{c8}")
                wqk_sb.append(w)
            wv_sb = []
            for c8 in range(8):
                w = cpool.tile([128, 256], F16, tag=f"wv{c8}", name=f"wv{c8}")
                wv_sb.append(w)
            wp_sb = []
            for k2 in range(2):
                w = cpool.tile([128, C], F16, tag=f"wp{k2}", name=f"wp{k2}")
                wp_sb.append(w)
            cost_sb = cpool.tile([128, T], F16, tag="cost", name="cost_sb")
            sint_sb = cpool.tile([128, T], F16, tag="sint", name="sint_sb")
            tri_sb = cpool.tile([128, 128], F16, tag="tri", name="tri_sb")
            ident_sb = cpool.tile([128, 128], F16, tag="ident", name="ident_sb")
            ones_sb = cpool.tile([128, 64], F16, tag="ones", name="ones_sb")
            rope_q = [cpool.tile([128, T], F16, tag=f"rq{p}", name=f"rq{p}")
                      for p in range(2)]
            rope_k = [cpool.tile([128, T], F16, tag=f"rk{p}", name=f"rk{p}")
                      for p in range(2)]
            vaug = cpool.tile([128, NBLK, 4, 65], F16, tag="vaug",
                              name="vaug")

            def load_consts_a():  # needed by the first q/k matmuls
                for c8 in range(8):
                    nc.sync.dma_start(out=wqk_sb[c8][:, :],
                                      in_=w_qk[c8 * 128:(c8 + 1) * 128, :])

            def load_consts_b():  # needed by rope / v / diag of att(0)
                for c8 in range(8):
                    nc.sync.dma_start(out=wv_sb[c8][:, :],
                                      in_=w_v[c8 * 128:(c8 + 1) * 128, :])
                nc.sync.dma_start(out=cost_sb[:, :], in_=cost_c[:, :])
                nc.sync.dma_start(out=sint_sb[:, :], in_=sint_c[:, :])
                nc.sync.dma_start(out=tri_sb[:, :], in_=tri_c[:, :])
                nc.sync.dma_start(out=ident_sb[:, :], in_=ident_c[:, :])
                nc.sync.dma_start(out=ones_sb[:, :], in_=ones_c[:, :])
                nc.vector.memset(vaug[:, :, :, 64], 1.0)

            def load_consts_c():  # needed from OP(0) on
                for k2 in range(2):
                    nc.sync.dma_start(
                        out=wp_sb[k2][:, :],
                        in_=w_p[k2 * 128:(k2 + 1) * 128, :])

            # ---- QKV pieces -------------------------------------------
            def qkv_pieces(c):
                t0 = c * TCH
                xts = []

                def load_x():
                    for c8 in range(8):
                        xt = xpool.tile([128, TCH], F16, tag="xt", name="xt")
                        nc.sync.dma_start(
                            out=xt[:, :],
                            in_=x_t[c8 * 128:(c8 + 1) * 128, t0:t0 + TCH])
                        xts.append(xt)

                def qk_piece(p, base, rope_dst):
                    ps = mmpool.tile([128, TCH], F32, tag="mm", name="qkps")
                    for c8 in range(8):
                        nc.tensor.matmul(
                            ps[:, :], wqk_sb[c8][:, base:base + 128],
                            xts[c8][:, :], start=(c8 == 0), stop=(c8 == 7))
                    m_sb = tpool.tile([128, TCH], F16, tag="msb", name="msb")
                    nc.vector.tensor_copy(m_sb[:, :], ps[:, :])
                    ms_sb = tpool.tile([128, TCH], F16, tag="mssb",
                                       name="mssb")
                    nc.sync.dma_start(out=ms_sb[0::2, :], in_=m_sb[1::2, :])
                    nc.sync.dma_start(out=ms_sb[1::2, :], in_=m_sb[0::2, :])
                    t1 = tpool.tile([128, TCH], F16, tag="t1", name="t1")
                    nc.gpsimd.tensor_mul(t1[:, :], m_sb[:, :],
                                         cost_sb[:, t0:t0 + TCH])
                    t2 = tpool.tile([128, TCH], F16, tag="t2", name="t2")
                    nc.gpsimd.tensor_mul(t2[:, :], ms_sb[:, :],
                                         sint_sb[:, t0:t0 + TCH])
                    nc.gpsimd.tensor_add(rope_dst[:, t0:t0 + TCH],
                                         t1[:, :], t2[:, :])

                def v_piece(j):  # two tk blocks per piece
                    vps = mmpool.tile([128, 2, 4, 64], F32, tag="mm",
                                      name="vps")
                    for i in range(2):
                        blk = 2 * j + i
                        for c8 in range(8):
                            nc.tensor.matmul(
                                vps[:, i, :, :],
                                xts[c8][:, 128 * blk:128 * blk + 128],
                                wv_sb[c8][:, :],
                                start=(c8 == 0), stop=(c8 == 7))
                    tb = 4 * c + 2 * j
                    nc.vector.tensor_copy(vaug[:, tb:tb + 2, :, 0:64],
                                          vps[:, :, :, :])

                pieces = [load_x]
                for p in range(2):
                    pieces.append(lambda p=p: qk_piece(p, 128 * p, rope_q[p]))
                    pieces.append(
                        lambda p=p: qk_piece(p, 256 + 128 * p, rope_k[p]))
                pieces.append(lambda: v_piece(0))
                pieces.append(lambda: v_piece(1))
                return pieces

            # ---- attention --------------------------------------------
            y_tiles = [None, None]

            def att_groups(g):
                """Bundles: per pair, per-group scores+exp (lag-1 AV),
                then normalize."""
                t0 = g * TCH
                bundles = []
                for p in range(2):
                    yps = [None, None]

                    def kq(h, Tt, c0, p=p):
                        return (rope_k[p][64 * h:64 * h + 64,
                                          128 * Tt:128 * Tt + 128],
                                rope_q[p][64 * h:64 * h + 64,
                                          t0 + c0:t0 + TCH])

                    def tri_mm(s2, off, stop=True):
                        nc.tensor.matmul(s2[:, off:off + 128],
                                         ident_sb[:, :], tri_sb[:, :],
                                         start=False, stop=stop)

                    def nd_group(Tt, p=p, yps=yps, kq=kq):
                        # full tile: h0 @0:512 (bank0), h1 @512:1024
                        s2 = spool.tile([128, 2 * TCH], F32, tag="s",
                                        name="s2")
                        for h in range(2):
                            kk, qq = kq(h, Tt, 0)
                            nc.tensor.matmul(s2[:, 512 * h:512 * h + 512],
                                             kk, qq, start=True, stop=True)
                        e = epool.tile([128, 2 * TCH], F16, tag="e", name="e")
                        nc.scalar.activation(e[:, :], s2[:, :], EXP,
                                             scale=0.125)

                        def av(p=p, yps=yps, e=e, Tt=Tt):
                            for h in range(2):
                                nc.tensor.matmul(
                                    yps[h][:, :],
                                    vaug[:, Tt, 2 * p + h, :],
                                    e[:, 512 * h:512 * h + 512],
                                    start=(Tt == 0), stop=False)
                        return av

                    def r0_group(p=p, yps=yps, kq=kq, tri_mm=tri_mm):
                        Tt = 4 * g
                        s2 = spool.tile([128, 2 * TCH], F32, tag="s",
                                        name="s2r0")
                        for h in range(2):
                            kk, qq = kq(h, Tt, 0)
                            nc.tensor.matmul(s2[:, 512 * h:512 * h + 512],
                                             kk, qq, start=True, stop=False)
                        for h in range(2):
                            tri_mm(s2, 512 * h)
                        e = epool.tile([128, 2 * TCH], F16, tag="e",
                                       name="er0")
                        if debug and g == 0 and p == 0:
                            s32 = smpool.tile([128, 2 * TCH], F32, tag="s32",
                                              name="s32")
                            nc.vector.tensor_copy(s32[:, :], s2[:, :])
                            nc.sync.dma_start(out=dbg["s0"][:, :],
                                              in_=s32[:, :])
                        nc.scalar.activation(e[:, :], s2[:, :], EXP,
                                             scale=0.125)
                        if debug and g == 0 and p == 0:
                            nc.sync.dma_start(out=dbg["e0"][:, :], in_=e[:, :])

                        def av(p=p, yps=yps, e=e, Tt=Tt):
                            for h in range(2):
                                nc.tensor.matmul(
                                    yps[h][:, :],
                                    vaug[:, Tt, 2 * p + h, :],
                                    e[:, 512 * h:512 * h + 512],
                                    start=(g == 0), stop=False)
                        return av

                    def r1_group(p=p, yps=yps, kq=kq, tri_mm=tri_mm):
                        # h0 @0:384 (bank0), dead 384:512, h1 @512:896
                        Tt = 4 * g + 1
                        s2 = spool.tile([128, 2 * TCH], F32, tag="s",
                                        name="s2r1")
                        nc.vector.memset(s2[:, 384:512], NEG)
                        for h in range(2):
                            kk, qq = kq(h, Tt, 128)
                            nc.tensor.matmul(s2[:, 512 * h:512 * h + 384],
                                             kk, qq, start=True, stop=False)
                        for h in range(2):
                            tri_mm(s2, 512 * h)
                        e = epool.tile([128, 2 * TCH], F16, tag="e",
                                       name="er1")
                        nc.scalar.activation(e[:, 0:896], s2[:, 0:896], EXP,
                                             scale=0.125)

                        def av(p=p, yps=yps, e=e, Tt=Tt):
                            for h in range(2):
                                nc.tensor.matmul(
                                    yps[h][:, 128:TCH],
                                    vaug[:, Tt, 2 * p + h, :],
                                    e[:, 512 * h:512 * h + 384],
                                    start=False, stop=False)
                        return av

                    def d2_group(p=p, yps=yps, kq=kq, tri_mm=tri_mm):
                        # h0: r2 @0:256, r3 @256:384 (bank0); dead 384:512
                        # h1: r2 @512:768, r3 @768:896 (bank1)
                        s2 = spool.tile([128, 2 * TCH], F32, tag="s",
                                        name="s2d2")
                        nc.vector.memset(s2[:, 384:512], NEG)
                        for h in range(2):
                            kk, qq = kq(h, 4 * g + 2, 256)
                            nc.tensor.matmul(s2[:, 512 * h:512 * h + 256],
                                             kk, qq, start=True, stop=False)
                        for h in range(2):
                            kk, qq = kq(h, 4 * g + 3, 384)
                            nc.tensor.matmul(
                                s2[:, 512 * h + 256:512 * h + 384],
                                kk, qq, start=False, stop=False)
                        for h in range(2):
                            tri_mm(s2, 512 * h, stop=False)
                            tri_mm(s2, 512 * h + 256)
                        e = epool.tile([128, 2 * TCH], F16, tag="e",
                                       name="ed2")
                        nc.scalar.activation(e[:, 0:896], s2[:, 0:896], EXP,
                                             scale=0.125)

                        def av(p=p, yps=yps, e=e):
                            for h in range(2):
                                hb = 512 * h
                                nc.tensor.matmul(
                                    yps[h][:, 256:TCH],
                                    vaug[:, 4 * g + 2, 2 * p + h, :],
                                    e[:, hb:hb + 256],
                                    start=False, stop=False)
                                nc.tensor.matmul(
                                    yps[h][:, 384:TCH],
                                    vaug[:, 4 * g + 3, 2 * p + h, :],
                                    e[:, hb + 256:hb + 384],
                                    start=False, stop=True)
                        return av

                    def normalize(p=p, yps=yps):
                        y_sb = ypool.tile([128, TCH], F16, tag=f"ysb{p}",
                                          name=f"ysb{p}")
                        if debug and g == 0 and p == 0:
                            y32 = smpool.tile([65, TCH], F32, tag="y32",
                                              name="y32")
                            nc.vector.tensor_copy(y32[:, :], yps[0][:, :])
                            nc.sync.dma_start(out=dbg["yraw"][:, :],
                                              in_=y32[:, :])
                        for h in range(2):
                            den16 = smpool.tile([65, TCH], F16, tag="den16",
                                                name="den16")
                            nc.scalar.activation(den16[64:65, :],
                                                 yps[h][64:65, :], CPY,
                                                 scale=0.0625)
                            dbc = mmpool.tile([128, TCH], F32, tag="mm",
                                              name="dbc")
                            nc.tensor.matmul(dbc[0:64, :],
                                             ones_sb[64:65, :],
                                             den16[64:65, :], start=True,
                                             stop=True)
                            rec = smpool.tile([64, TCH], F32, tag="rec",
                                              name="rec")
                            scr = smpool.tile([64, TCH], F32, tag="scr",
                                              name="scr")
                            nc.vector.reciprocal_approx_accurate(
                                rec[:, :], dbc[0:64, :], scr[:, :])
                            nc.vector.scalar_tensor_tensor(
                                y_sb[64 * h:64 * h + 64, :],
                                yps[h][0:64, :], 0.0625, rec[:, :],
                                op0=MUL, op1=MUL)
                            if debug and g == 0 and p == 0 and h == 0:
                                nc.sync.dma_start(out=dbg["rec"][:, :],
                                                  in_=rec[0:1, :])
                        if debug and g == 0 and p == 0:
                            nc.sync.dma_start(out=dbg["ysb"][:, :],
                                              in_=y_sb[:, :])
                        y_tiles[p] = y_sb

                    state = {"av": None}
                    grp_fns = [lambda Tt=Tt, f=nd_group: f(Tt)
                               for Tt in range(4 * g)]
                    grp_fns += [r0_group, r1_group, d2_group]

                    def piped(spec, first=False, yps=yps, state=state):
                        def run():
                            if first:
                                for h in range(2):
                                    yps[h] = ypsum.tile(
                                        [65, TCH], F32, tag=f"y{h}",
                                        name=f"y{h}")
                            av = spec()
                            prev = state["av"]
                            state["av"] = av
                            if prev:
                                prev()
                        return run

                    def tail(fn=normalize, state=state):
                        def run():
                            state["av"]()
                            state["av"] = None
                            fn()
                        return run

                    bundles.append(piped(grp_fns[0], first=True))
                    bundles.extend(piped(f) for f in grp_fns[1:])
                    bundles.append(tail())
                return bundles

            # ---- output projection + collectives ----------------------
            def op_pieces(g):
                """8 m-tile pieces with a half ReduceScatter after each
                4; captures the y tiles at call time."""
                y0, y1 = y_tiles[0], y_tiles[1]
                pieces = []

                def m_piece(m8, y0=y0, y1=y1):
                    op_ps = mmpool.tile([128, TCH], F32, tag="mm",
                                        name="opps")
                    nc.tensor.matmul(op_ps[:, :],
                                     wp_sb[0][:, 128 * m8:128 * m8 + 128],
                                     y0[:, :], start=True, stop=False)
                    nc.tensor.matmul(op_ps[:, :],
                                     wp_sb[1][:, 128 * m8:128 * m8 + 128],
                                     y1[:, :], start=False, stop=True)
                    o_sb = opool.tile([128, TCH], F16, tag="osb", name="osb")
                    if m8 % 2 == 0:
                        nc.vector.tensor_copy(o_sb[:, :], op_ps[:, :])
                    else:
                        nc.scalar.copy(o_sb[:, :], op_ps[:, :])
                    nc.sync.dma_start(
                        out=cc_in[g][128 * m8:128 * m8 + 128, :],
                        in_=o_sb[:, :])

                def rs_half(hh):
                    nc.gpsimd.collective_compute(
                        "ReduceScatter", mybir.AluOpType.add,
                        replica_groups=groups,
                        ins=[cc_in[g][512 * hh:512 * (hh + 1), :].opt()],
                        outs=[cc_out[g][hh].ap().opt()])
                    nc.sync.dma_start(out=out_ext[g, hh, :, :],
                                      in_=cc_out[g][hh][:, :])

                for m8 in range(8):
                    pieces.append(lambda m8=m8: m_piece(m8))
                    if m8 == 3:
                        pieces.append(lambda: rs_half(0))
                pieces.append(lambda: rs_half(1))
                return pieces

            def interleave(bundles, fillers):
                nb, nf = len(bundles), len(fillers)
                fi = 0
                for i, bnd in enumerate(bundles):
                    bnd()
                    want = (i + 1) * nf // nb
                    while fi < want:
                        fillers[fi]()
                        fi += 1
                while fi < nf:
                    fillers[fi]()
                    fi += 1

            # ---- main emission ----------------------------------------
            load_consts_a()
            qkv0 = qkv_pieces(0)
            qkv0[0]()          # x chunk-0 loads right after w_qk
            load_consts_b()
            for f in qkv0[1:]:
                f()
            load_consts_c()
            pending_op = []
            for g in range(NTC):
                fillers = []
                if g + 1 < NTC:
                    fillers += qkv_pieces(g + 1)
                fillers += pending_op
                interleave(att_groups(g), fillers)
                pending_op = op_pieces(g)
            for f in pending_op:
                f()

    if not nc.is_finalized():
        nc.finalize()
    return nc


_NC_CACHE = None


def _get_nc():
    global _NC_CACHE
    if _NC_CACHE is None:
        _NC_CACHE = build()
    return _NC_CACHE


def make_in_maps(x, w_qkv, w_proj):
    x = np.asarray(x, np.float32)
    w_qkv = np.asarray(w_qkv, np.float32)
    w_proj = np.asarray(w_proj, np.float32)
    x_tb = [np.ascontiguousarray(x[b].T).astype(np.float16)
            for b in range(B)]
    maps = []
    for r in range(NCORE):
        b, q = divmod(r, 4)
        heads = [4 * q + i for i in range(4)]
        qcols, kcols = [], []
        for p in range(2):
            hs = heads[2 * p:2 * p + 2]
            rows = [h * 64 + d for h in hs for d in range(D)]
            qcols.append(w_qkv[rows, :].T)
            kcols.append(w_qkv[[C + i for i in rows], :].T)
        w_qk = np.ascontiguousarray(
            np.concatenate(qcols + kcols, axis=1)).astype(np.float16)
        vrows = [2 * C + h * 64 + d for h in heads for d in range(D)]
        w_v = np.ascontiguousarray(w_qkv[vrows, :].T).astype(np.float16)
        mydims = [h * 64 + d for h in heads for d in range(D)]
        w_p = np.ascontiguousarray(w_proj[:, mydims].T).astype(np.float16)
        maps.append({"x_t": x_tb[b], "w_qk": w_qk, "w_v": w_v, "w_p": w_p})
    return maps


def assemble(results):
    outT = np.zeros((B, C, T), np.float32)
    for r in range(NCORE):
        b, q = divmod(r, 4)
        o = results[r]["out"].astype(np.float32)  # [4, 2, 128, TCH]
        for g in range(NTC):
            for hh in range(2):
                r0 = 512 * hh + 128 * q
                outT[b, r0:r0 + 128, g * TCH:(g + 1) * TCH] = o[g, hh]
    return np.ascontiguousarray(outT.transpose(0, 2, 1))


def run(x, w_qkv, w_proj, trace=False):
    nc = _get_nc()
    in_maps = make_in_maps(x, w_qkv, w_proj)
    res = run_bass_kernel_spmd(nc, in_maps, list(range(NCORE)), trace=trace)
    return assemble(res.results), res


def kernel(x, w_qkv, w_proj):
    out, _ = run(x, w_qkv, w_proj, trace=False)
    return out


# revision 45
# speedup vs baseline: 1.3492x; 1.0673x over previous
"""Distributed causal attention w/ RoPE for TRN2 (8 NeuronCores).

Sharding: 2-way batch x 4-way head-group. Core r: batch b=r//4, quad
q=r%4, heads 4q..4q+3 as two pairs. Per core:
  - QKV projection of its batch only. q/k in transposed pair layout
    [128=2*64 dims, T] with fused RoPE (pair-swap via strided DMA, mul/add
    on GpSimd). v projected directly in natural [tk, d] layout (x tile as
    stationary operand) -- no transposes; a ones column is appended per
    head so the AV matmul also produces the softmax denominator (M=65).
  - Scores computed transposed [tk, tq]: one tk-tile x 2 heads per
    2-bank PSUM tile [128, 1024], double-buffered so score matmuls for
    group i+1 overlap the exp of group i. The two heads' score matmuls
    use disjoint PE row groups (K=64 at base partitions 0/64) and run
    concurrently. One exp ACTIVATE per group with the 1/8 softmax scale
    folded into the activation scale; causal triangle applied as an
    accumulated identity x (-30000 mask) matmul on the PE.
  - Software pipelining: group i's AV matmuls are emitted after group
    i+1's scores+exp, so the PE never waits on ACT.
  - Normalize: den row (partition 64) scaled-copy to f16, broadcast to
    partitions 0..63 with a K=1 ones matmul at row offset 64, reciprocal
    at base partition 0 (custom DVE ops ignore AP base partition), one
    scalar_tensor_tensor multiply.
  - Output projection partials (K=256) -> per-half ReduceScatters within
    the 4-core batch group, overlapped with the next query group.
"""

import numpy as np

import concourse.bass as bass
import concourse.bacc as bacc
import concourse.mybir as mybir
from concourse import tile
from concourse.bass_utils import run_bass_kernel_spmd

B, T, C, H, D = 2, 2048, 1024, 16, 64
NCORE = 8
TCH = 512                 # token chunk (query group)
NTC = T // TCH            # 4
NBLK = T // 128           # 16 tk blocks
ROPE_BASE = 10000.0
NEG = -30000.0
F32 = mybir.dt.float32
F16 = mybir.dt.float16
MUL = mybir.AluOpType.mult
EXP = mybir.ActivationFunctionType.Exp
CPY = mybir.ActivationFunctionType.Copy


def _rope_tables():
    d = np.arange(D)
    j = d // 2
    theta = ROPE_BASE ** (-(2.0 * j) / D)
    t = np.arange(T, dtype=np.float64)
    ang = t[None, :] * theta[:, None]
    cos = np.cos(ang)
    sin = np.sin(ang)
    sgn = np.where(d % 2 == 0, -1.0, 1.0)[:, None]
    c1 = np.concatenate([cos, cos], axis=0).astype(np.float16)
    s1 = np.concatenate([sgn * sin, sgn * sin], axis=0).astype(np.float16)
    return c1, s1


def _tri():
    tk = np.arange(128)[:, None]
    jj = np.arange(128)[None, :]
    return np.where(jj >= tk, 0.0, NEG).astype(np.float16)


def build(debug=False):
    nc = bacc.Bacc(num_devices=NCORE)
    x_t = nc.declare_dram_parameter("x_t", [C, T], F16, isOutput=False)
    w_qk = nc.declare_dram_parameter("w_qk", [C, 512], F16, isOutput=False)
    w_v = nc.declare_dram_parameter("w_v", [C, 256], F16, isOutput=False)
    w_p = nc.declare_dram_parameter("w_p", [256, C], F16, isOutput=False)
    out_ext = nc.declare_dram_parameter("out", [NTC, 256, TCH], F16,
                                        isOutput=True)
    dbg = {}
    if debug:
        dbg["e0"] = nc.declare_dram_parameter("dbg_e0", [128, 1024], F16,
                                              isOutput=True)
        dbg["s0"] = nc.declare_dram_parameter("dbg_s0", [128, 1024], F32,
                                              isOutput=True)
        dbg["yraw"] = nc.declare_dram_parameter("dbg_yraw", [65, TCH], F32,
                                                isOutput=True)
        dbg["rec"] = nc.declare_dram_parameter("dbg_rec", [1, TCH], F32,
                                               isOutput=True)
        dbg["ysb"] = nc.declare_dram_parameter("dbg_ysb", [128, TCH], F16,
                                               isOutput=True)

    c1_np, s1_np = _rope_tables()
    cost_c = nc.inline_tensor(c1_np, name="cost")
    sint_c = nc.inline_tensor(s1_np, name="sint")
    tri_c = nc.inline_tensor(_tri(), name="tri")
    ident_c = nc.inline_tensor(np.eye(128, dtype=np.float16), name="ident")
    ones_c = nc.inline_tensor(np.ones((128, 64), np.float16), name="ones1")

    cc_in = [nc.dram_tensor(f"cc_in{g}", [C, TCH], F16) for g in range(NTC)]
    cc_out = [nc.dram_tensor(f"cc_out{g}", [256, TCH], F16)
              for g in range(NTC)]
    groups = [[0, 1, 2, 3], [4, 5, 6, 7]]

    with tile.TileContext(nc) as tc:
        with (
            tc.tile_pool(name="spsum", bufs=2, space="PSUM") as spool,
            tc.tile_pool(name="ypsum", bufs=1, space="PSUM") as ypsum,
            tc.tile_pool(name="mm", bufs=2, space="PSUM") as mmpool,
            tc.tile_pool(name="const", bufs=1) as cpool,
            tc.tile_pool(name="xt", bufs=16) as xpool,
            tc.tile_pool(name="tmp", bufs=6) as tpool,
            tc.tile_pool(name="exp", bufs=4) as epool,
            tc.tile_pool(name="ysb", bufs=4) as ypool,
            tc.tile_pool(name="osb", bufs=4) as opool,
            tc.tile_pool(name="small", bufs=4) as smpool,
        ):
            # ---- persistent SBUF tiles --------------------------------
            # (x chunk-0 + w_qk loads go first on the sync queue; wv and
            # the rope tables on the vector queue; wp/tri/ident/ones on
            # the scalar queue -- parallel DMA rings, and nothing the
            # first matmuls need is queued behind cold constants.)
            wqk_sb = []
            for c8 in range(8):
                w = cpool.tile([128, 512], F16, tag=f"wqk{c8}", name=f"wqk{c8}")
                wqk_sb.append(w)
            wv_sb = []
            for c8 in range(8):
                w = cpool.tile([128, 256], F16, tag=f"wv{c8}", name=f"wv{c8}")
                wv_sb.append(w)
            wp_sb = []
            for k2 in range(2):
                w = cpool.tile([128, C], F16, tag=f"wp{k2}", name=f"wp{k2}")
                wp_sb.append(w)
            cost_sb = cpool.tile([128, T], F16, tag="cost", name="cost_sb")
            sint_sb = cpool.tile([128, T], F16, tag="sint", name="sint_sb")
            tri_sb = cpool.tile([128, 128], F16, tag="tri", name="tri_sb")
            ident_sb = cpool.tile([128, 128], F16, tag="ident", name="ident_sb")
            ones_sb = cpool.tile([128, 64], F16, tag="ones", name="ones_sb")
            rope_q = [cpool.tile([128, T], F16, tag=f"rq{p}", name=f"rq{p}")
                      for p in range(2)]
            rope_k = [cpool.tile([128, T], F16, tag=f"rk{p}", name=f"rk{p}")
                      for p in range(2)]
            vaug = cpool.tile([128, NBLK, 4, 65], F16, tag="vaug",
                              name="vaug")

            def load_consts_a():  # needed by the first q/k matmuls + rope
                for c8 in range(8):
                    nc.sync.dma_start(out=wqk_sb[c8][:, :],
                                      in_=w_qk[c8 * 128:(c8 + 1) * 128, :])
                nc.sync.dma_start(out=cost_sb[:, :], in_=cost_c[:, :])
                nc.sync.dma_start(out=sint_sb[:, :], in_=sint_c[:, :])

            def load_consts_b():  # needed by v / diag of att(0)
                for c8 in range(8):
                    nc.sync.dma_start(out=wv_sb[c8][:, :],
                                      in_=w_v[c8 * 128:(c8 + 1) * 128, :])
                nc.sync.dma_start(out=tri_sb[:, :], in_=tri_c[:, :])
                nc.sync.dma_start(out=ident_sb[:, :], in_=ident_c[:, :])
                nc.sync.dma_start(out=ones_sb[:, :], in_=ones_c[:, :])
                nc.vector.memset(vaug[:, :, :, 64], 1.0)

            def load_consts_c():  # needed from OP(0) on
                for k2 in range(2):
                    nc.sync.dma_start(
                        out=wp_sb[k2][:, :],
                        in_=w_p[k2 * 128:(k2 + 1) * 128, :])

            # ---- QKV pieces -------------------------------------------
            def qkv_pieces(c):
                t0 = c * TCH
                xts = []

                def load_x():
                    for c8 in range(8):
                        xt = xpool.tile([128, TCH], F16, tag="xt", name="xt")
                        nc.sync.dma_start(
                            out=xt[:, :],
                            in_=x_t[c8 * 128:(c8 + 1) * 128, t0:t0 + TCH])
                        xts.append(xt)

                def qk_piece(p, base, rope_dst):
                    ps = mmpool.tile([128, TCH], F32, tag="mm", name="qkps")
                    for c8 in range(8):
                        nc.tensor.matmul(
                            ps[:, :], wqk_sb[c8][:, base:base + 128],
                            xts[c8][:, :], start=(c8 == 0), stop=(c8 == 7))
                    m_sb = tpool.tile([128, TCH], F16, tag="msb", name="msb")
                    nc.vector.tensor_copy(m_sb[:, :], ps[:, :])
                    ms_sb = tpool.tile([128, TCH], F16, tag="mssb",
                                       name="mssb")
                    nc.sync.dma_start(out=ms_sb[0::2, :], in_=m_sb[1::2, :])
                    nc.sync.dma_start(out=ms_sb[1::2, :], in_=m_sb[0::2, :])
                    t1 = tpool.tile([128, TCH], F16, tag="t1", name="t1")
                    nc.vector.tensor_mul(t1[:, :], m_sb[:, :],
                                         cost_sb[:, t0:t0 + TCH])
                    t2 = tpool.tile([128, TCH], F16, tag="t2", name="t2")
                    nc.vector.tensor_mul(t2[:, :], ms_sb[:, :],
                                         sint_sb[:, t0:t0 + TCH])
                    nc.vector.tensor_add(rope_dst[:, t0:t0 + TCH],
                                         t1[:, :], t2[:, :])

                def v_piece(j):  # two tk blocks per piece
                    vps = mmpool.tile([128, 2, 4, 64], F32, tag="mm",
                                      name="vps")
                    for i in range(2):
                        blk = 2 * j + i
                        for c8 in range(8):
                            nc.tensor.matmul(
                                vps[:, i, :, :],
                                xts[c8][:, 128 * blk:128 * blk + 128],
                                wv_sb[c8][:, :],
                                start=(c8 == 0), stop=(c8 == 7))
                    tb = 4 * c + 2 * j
                    nc.vector.tensor_copy(vaug[:, tb:tb + 2, :, 0:64],
                                          vps[:, :, :, :])

                pieces = [load_x]
                for p in range(2):
                    pieces.append(lambda p=p: qk_piece(p, 128 * p, rope_q[p]))
                    pieces.append(
                        lambda p=p: qk_piece(p, 256 + 128 * p, rope_k[p]))
                pieces.append(lambda: v_piece(0))
                pieces.append(lambda: v_piece(1))
                return pieces

            # ---- attention --------------------------------------------
            y_tiles = [None, None]

            def att_groups(g):
                """Bundles: per pair, per-group scores+exp (lag-1 AV),
                then normalize."""
                t0 = g * TCH
                bundles = []
                for p in range(2):
                    yps = [None, None]

                    def kq(h, Tt, c0, p=p):
                        return (rope_k[p][64 * h:64 * h + 64,
                                          128 * Tt:128 * Tt + 128],
                                rope_q[p][64 * h:64 * h + 64,
                                          t0 + c0:t0 + TCH])

                    def tri_mm(s2, off, stop=True):
                        nc.tensor.matmul(s2[:, off:off + 128],
                                         ident_sb[:, :], tri_sb[:, :],
                                         start=False, stop=stop)

                    def nd_group(Tt, p=p, yps=yps, kq=kq):
                        # full tile: h0 @0:512 (bank0), h1 @512:1024
                        s2 = spool.tile([128, 2 * TCH], F32, tag="s",
                                        name="s2")
                        for h in range(2):
                            kk, qq = kq(h, Tt, 0)
                            nc.tensor.matmul(s2[:, 512 * h:512 * h + 512],
                                             kk, qq, start=True, stop=True)
                        e = epool.tile([128, 2 * TCH], F16, tag="e", name="e")
                        nc.scalar.activation(e[:, :], s2[:, :], EXP,
                                             scale=0.125)

                        def av(p=p, yps=yps, e=e, Tt=Tt):
                            for h in range(2):
                                nc.tensor.matmul(
                                    yps[h][:, :],
                                    vaug[:, Tt, 2 * p + h, :],
                                    e[:, 512 * h:512 * h + 512],
                                    start=(Tt == 0), stop=False)
                        return av

                    def r0_group(p=p, yps=yps, kq=kq, tri_mm=tri_mm):
                        Tt = 4 * g
                        s2 = spool.tile([128, 2 * TCH], F32, tag="s",
                                        name="s2r0")
                        for h in range(2):
                            kk, qq = kq(h, Tt, 0)
                            nc.tensor.matmul(s2[:, 512 * h:512 * h + 512],
                                             kk, qq, start=True, stop=False)
                        for h in range(2):
                            tri_mm(s2, 512 * h)
                        e = epool.tile([128, 2 * TCH], F16, tag="e",
                                       name="er0")
                        if debug and g == 0 and p == 0:
                            s32 = smpool.tile([128, 2 * TCH], F32, tag="s32",
                                              name="s32")
                            nc.vector.tensor_copy(s32[:, :], s2[:, :])
                            nc.sync.dma_start(out=dbg["s0"][:, :],
                                              in_=s32[:, :])
                        nc.scalar.activation(e[:, :], s2[:, :], EXP,
                                             scale=0.125)
                        if debug and g == 0 and p == 0:
                            nc.sync.dma_start(out=dbg["e0"][:, :], in_=e[:, :])

                        def av(p=p, yps=yps, e=e, Tt=Tt):
                            for h in range(2):
                                nc.tensor.matmul(
                                    yps[h][:, :],
                                    vaug[:, Tt, 2 * p + h, :],
                                    e[:, 512 * h:512 * h + 512],
                                    start=(g == 0), stop=False)
                        return av

                    def r1_group(p=p, yps=yps, kq=kq, tri_mm=tri_mm):
                        # h0 @0:384 (bank0), dead 384:512, h1 @512:896
                        Tt = 4 * g + 1
                        s2 = spool.tile([128, 2 * TCH], F32, tag="s",
                                        name="s2r1")
                        nc.vector.memset(s2[:, 384:512], NEG)
                        for h in range(2):
                            kk, qq = kq(h, Tt, 128)
                            nc.tensor.matmul(s2[:, 512 * h:512 * h + 384],
                                             kk, qq, start=True, stop=False)
                        for h in range(2):
                            tri_mm(s2, 512 * h)
                        e = epool.tile([128, 2 * TCH], F16, tag="e",
                                       name="er1")
                        nc.scalar.activation(e[:, 0:896], s2[:, 0:896], EXP,
                                             scale=0.125)

                        def av(p=p, yps=yps, e=e, Tt=Tt):
                            for h in range(2):
                                nc.tensor.matmul(
                                    yps[h][:, 128:TCH],
                                    vaug[:, Tt, 2 * p + h, :],
                                    e[:, 512 * h:512 * h + 384],
                                    start=False, stop=False)
                        return av

                    def d2_group(p=p, yps=yps, kq=kq, tri_mm=tri_mm):
                        # h0: r2 @0:256, r3 @256:384 (bank0); dead 384:512
                        # h1: r2 @512:768, r3 @768:896 (bank1)
                        s2 = spool.tile([128, 2 * TCH], F32, tag="s",
                                        name="s2d2")
                        nc.vector.memset(s2[:, 384:512], NEG)
                        for h in range(2):
                            kk, qq = kq(h, 4 * g + 2, 256)
                            nc.tensor.matmul(s2[:, 512 * h:512 * h + 256],
                                             kk, qq, start=True, stop=False)
                        for h in range(2):
                            kk, qq = kq(h, 4 * g + 3, 384)
                            nc.tensor.matmul(
                                s2[:, 512 * h + 256:512 * h + 384],
                                kk, qq, start=False, stop=False)
                        for h in range(2):
                            tri_mm(s2, 512 * h, stop=False)
                            tri_mm(s2, 512 * h + 256)
                        e = epool.tile([128, 2 * TCH], F16, tag="e",
                                       name="ed2")
                        nc.scalar.activation(e[:, 0:896], s2[:, 0:896], EXP,
                                             scale=0.125)

                        def av(p=p, yps=yps, e=e):
                            for h in range(2):
                                hb = 512 * h
                                nc.tensor.matmul(
                                    yps[h][:, 256:TCH],
                                    vaug[:, 4 * g + 2, 2 * p + h, :],
                                    e[:, hb:hb + 256],
                                    start=False, stop=False)
                                nc.tensor.matmul(
                                    yps[h][:, 384:TCH],
                                    vaug[:, 4 * g + 3, 2 * p + h, :],
                                    e[:, hb + 256:hb + 384],
                                    start=False, stop=True)
                        return av

                    def normalize(p=p, yps=yps):
                        y_sb = ypool.tile([128, TCH], F16, tag=f"ysb{p}",
                                          name=f"ysb{p}")
                        if debug and g == 0 and p == 0:
                            y32 = smpool.tile([65, TCH], F32, tag="y32",
                                              name="y32")
                            nc.vector.tensor_copy(y32[:, :], yps[0][:, :])
                            nc.sync.dma_start(out=dbg["yraw"][:, :],
                                              in_=y32[:, :])
                        for h in range(2):
                            den16 = smpool.tile([65, TCH], F16, tag="den16",
                                                name="den16")
                            nc.vector.tensor_copy(den16[64:65, :],
                                                  yps[h][64:65, :])
                            dbc = mmpool.tile([128, TCH], F32, tag="mm",
                                              name="dbc")
                            nc.tensor.matmul(dbc[0:64, :],
                                             ones_sb[64:65, :],
                                             den16[64:65, :], start=True,
                                             stop=True)
                            rec = smpool.tile([64, TCH], F32, tag="rec",
                                              name="rec")
                            nc.vector.reciprocal_approx_fast(
                                rec[:, :], dbc[0:64, :])
                            nc.vector.scalar_tensor_tensor(
                                y_sb[64 * h:64 * h + 64, :],
                                yps[h][0:64, :], 1.0, rec[:, :],
                                op0=MUL, op1=MUL)
                            if debug and g == 0 and p == 0 and h == 0:
                                nc.sync.dma_start(out=dbg["rec"][:, :],
                                                  in_=rec[0:1, :])
                        if debug and g == 0 and p == 0:
                            nc.sync.dma_start(out=dbg["ysb"][:, :],
                                              in_=y_sb[:, :])
                        y_tiles[p] = y_sb

                    state = {"av": None}
                    grp_fns = [lambda Tt=Tt, f=nd_group: f(Tt)
                               for Tt in range(4 * g)]
                    grp_fns += [r0_group, r1_group, d2_group]

                    def piped(spec, first=False, yps=yps, state=state):
                        def run():
                            if first:
                                for h in range(2):
                                    yps[h] = ypsum.tile(
                                        [65, TCH], F32, tag=f"y{h}",
                                        name=f"y{h}")
                            av = spec()
                            prev = state["av"]
                            state["av"] = av
                            if prev:
                                prev()
                        return run

                    def tail(fn=normalize, state=state):
                        def run():
                            state["av"]()
                            state["av"] = None
                            fn()
                        return run

                    bundles.append(piped(grp_fns[0], first=True))
                    bundles.extend(piped(f) for f in grp_fns[1:])
                    bundles.append(tail())
                return bundles

            # ---- output projection + collectives ----------------------
            def op_pieces(g):
                """8 m-tile pieces with a half ReduceScatter after each
                4; captures the y tiles at call time."""
                y0, y1 = y_tiles[0], y_tiles[1]
                pieces = []

                def m_piece(m8, y0=y0, y1=y1):
                    op_ps = mmpool.tile([128, TCH], F32, tag="mm",
                                        name="opps")
                    nc.tensor.matmul(op_ps[:, :],
                                     wp_sb[0][:, 128 * m8:128 * m8 + 128],
                                     y0[:, :], start=True, stop=False)
                    nc.tensor.matmul(op_ps[:, :],
                                     wp_sb[1][:, 128 * m8:128 * m8 + 128],
                                     y1[:, :], start=False, stop=True)
                    o_sb = opool.tile([128, TCH], F16, tag="osb", name="osb")
                    if m8 % 2 == 0:
                        nc.vector.tensor_copy(o_sb[:, :], op_ps[:, :])
                    else:
                        nc.scalar.copy(o_sb[:, :], op_ps[:, :])
                    nc.sync.dma_start(
                        out=cc_in[g][128 * m8:128 * m8 + 128, :],
                        in_=o_sb[:, :])

                def rs():
                    nc.gpsimd.collective_compute(
                        "ReduceScatter", mybir.AluOpType.add,
                        replica_groups=groups,
                        ins=[cc_in[g].ap().opt()],
                        outs=[cc_out[g].ap().opt()])
                    nc.sync.dma_start(out=out_ext[g, :, :],
                                      in_=cc_out[g][:, :])

                for m8 in range(8):
                    pieces.append(lambda m8=m8: m_piece(m8))
                pieces.append(rs)
                return pieces

            def interleave(bundles, fillers):
                nb, nf = len(bundles), len(fillers)
                fi = 0
                for i, bnd in enumerate(bundles):
                    bnd()
                    want = (i + 1) * nf // nb
                    while fi < want:
                        fillers[fi]()
                        fi += 1
                while fi < nf:
                    fillers[fi]()
                    fi += 1

            # ---- main emission ----------------------------------------
            load_consts_a()
            qkv0 = qkv_pieces(0)
            qkv0[0]()          # x chunk-0 loads right after w_qk
            load_consts_b()
            for f in qkv0[1:]:
                f()
            load_consts_c()
            pending_op = []
            for g in range(NTC):
                fillers = []
                if g + 1 < NTC:
                    fillers += qkv_pieces(g + 1)
                fillers += pending_op
                interleave(att_groups(g), fillers)
                pending_op = op_pieces(g)
            for f in pending_op:
                f()

    if not nc.is_finalized():
        nc.finalize()
    return nc


_NC_CACHE = None


def _get_nc():
    global _NC_CACHE
    if _NC_CACHE is None:
        _NC_CACHE = build()
    return _NC_CACHE


def make_in_maps(x, w_qkv, w_proj):
    x = np.asarray(x, np.float32)
    w_qkv = np.asarray(w_qkv, np.float32)
    w_proj = np.asarray(w_proj, np.float32)
    x_tb = [np.ascontiguousarray(x[b].T).astype(np.float16)
            for b in range(B)]
    maps = []
    for r in range(NCORE):
        b, q = divmod(r, 4)
        heads = [4 * q + i for i in range(4)]
        qcols, kcols = [], []
        for p in range(2):
            hs = heads[2 * p:2 * p + 2]
            rows = [h * 64 + d for h in hs for d in range(D)]
            qcols.append(w_qkv[rows, :].T)
            kcols.append(w_qkv[[C + i for i in rows], :].T)
        w_qk = np.ascontiguousarray(
            np.concatenate(qcols + kcols, axis=1)).astype(np.float16)
        vrows = [2 * C + h * 64 + d for h in heads for d in range(D)]
        w_v = np.ascontiguousarray(w_qkv[vrows, :].T).astype(np.float16)
        mydims = [h * 64 + d for h in heads for d in range(D)]
        w_p = np.ascontiguousarray(w_proj[:, mydims].T).astype(np.float16)
        maps.append({"x_t": x_tb[b], "w_qk": w_qk, "w_v": w_v, "w_p": w_p})
    return maps


def assemble(results):
    outT = np.zeros((B, C, T), np.float32)
    for r in range(NCORE):
        b, q = divmod(r, 4)
        o = results[r]["out"].astype(np.float32)  # [4, 256, TCH]
        for g in range(NTC):
            outT[b, 256 * q:256 * (q + 1), g * TCH:(g + 1) * TCH] = o[g]
    return np.ascontiguousarray(outT.transpose(0, 2, 1))


def run(x, w_qkv, w_proj, trace=False):
    nc = _get_nc()
    in_maps = make_in_maps(x, w_qkv, w_proj)
    res = run_bass_kernel_spmd(nc, in_maps, list(range(NCORE)), trace=trace)
    return assemble(res.results), res


def kernel(x, w_qkv, w_proj):
    out, _ = run(x, w_qkv, w_proj, trace=False)
    return out


# revision 47
# speedup vs baseline: 1.3980x; 1.0362x over previous
"""Distributed causal attention w/ RoPE for TRN2 (8 NeuronCores).

Sharding: 2-way batch x 4-way head-group. Core r: batch b=r//4, quad
q=r%4, heads 4q..4q+3 as two pairs. Per core:
  - QKV projection of its batch only. q/k in transposed pair layout
    [128=2*64 dims, T] with fused RoPE (pair-swap via strided DMA, mul/add
    on GpSimd). v projected directly in natural [tk, d] layout (x tile as
    stationary operand) -- no transposes; a ones column is appended per
    head so the AV matmul also produces the softmax denominator (M=65).
  - Scores computed transposed [tk, tq]: one tk-tile x 2 heads per
    2-bank PSUM tile [128, 1024], double-buffered so score matmuls for
    group i+1 overlap the exp of group i. The two heads' score matmuls
    use disjoint PE row groups (K=64 at base partitions 0/64) and run
    concurrently. One exp ACTIVATE per group with the 1/8 softmax scale
    folded into the activation scale; causal triangle applied as an
    accumulated identity x (-30000 mask) matmul on the PE.
  - Software pipelining: group i's AV matmuls are emitted after group
    i+1's scores+exp, so the PE never waits on ACT.
  - Normalize: den row (partition 64) scaled-copy to f16, broadcast to
    partitions 0..63 with a K=1 ones matmul at row offset 64, reciprocal
    at base partition 0 (custom DVE ops ignore AP base partition), one
    scalar_tensor_tensor multiply.
  - Output projection partials (K=256) -> per-half ReduceScatters within
    the 4-core batch group, overlapped with the next query group.
"""

import numpy as np

import concourse.bass as bass
import concourse.bacc as bacc
import concourse.mybir as mybir
from concourse import tile
from concourse.bass_utils import run_bass_kernel_spmd

B, T, C, H, D = 2, 2048, 1024, 16, 64
NCORE = 8
TCH = 512                 # token chunk (query group)
NTC = T // TCH            # 4
NBLK = T // 128           # 16 tk blocks
ROPE_BASE = 10000.0
NEG = -30000.0
F32 = mybir.dt.float32
F16 = mybir.dt.float16
MUL = mybir.AluOpType.mult
EXP = mybir.ActivationFunctionType.Exp
CPY = mybir.ActivationFunctionType.Copy


def _rope_tables():
    d = np.arange(D)
    j = d // 2
    theta = ROPE_BASE ** (-(2.0 * j) / D)
    t = np.arange(T, dtype=np.float64)
    ang = t[None, :] * theta[:, None]
    cos = np.cos(ang)
    sin = np.sin(ang)
    sgn = np.where(d % 2 == 0, -1.0, 1.0)[:, None]
    c1 = np.concatenate([cos, cos], axis=0).astype(np.float16)
    s1 = np.concatenate([sgn * sin, sgn * sin], axis=0).astype(np.float16)
    return c1, s1


def _tri():
    tk = np.arange(128)[:, None]
    jj = np.arange(128)[None, :]
    return np.where(jj >= tk, 0.0, NEG).astype(np.float16)


def build(debug=False):
    nc = bacc.Bacc(num_devices=NCORE)
    x_t = nc.declare_dram_parameter("x_t", [C, T], F16, isOutput=False)
    w_qk = nc.declare_dram_parameter("w_qk", [C, 512], F16, isOutput=False)
    w_v = nc.declare_dram_parameter("w_v", [C, 256], F16, isOutput=False)
    w_p = nc.declare_dram_parameter("w_p", [256, C], F16, isOutput=False)
    out_ext = nc.declare_dram_parameter("out", [NTC, 256, TCH], F16,
                                        isOutput=True)
    dbg = {}
    if debug:
        dbg["e0"] = nc.declare_dram_parameter("dbg_e0", [128, 1024], F16,
                                              isOutput=True)
        dbg["s0"] = nc.declare_dram_parameter("dbg_s0", [128, 1024], F32,
                                              isOutput=True)
        dbg["yraw"] = nc.declare_dram_parameter("dbg_yraw", [65, TCH], F32,
                                                isOutput=True)
        dbg["rec"] = nc.declare_dram_parameter("dbg_rec", [1, TCH], F32,
                                               isOutput=True)
        dbg["ysb"] = nc.declare_dram_parameter("dbg_ysb", [128, TCH], F16,
                                               isOutput=True)

    c1_np, s1_np = _rope_tables()
    cost_c = nc.inline_tensor(c1_np, name="cost")
    sint_c = nc.inline_tensor(s1_np, name="sint")
    tri_c = nc.inline_tensor(_tri(), name="tri")
    ident_c = nc.inline_tensor(np.eye(128, dtype=np.float16), name="ident")
    ones_c = nc.inline_tensor(np.ones((128, 64), np.float16), name="ones1")

    cc_in = [nc.dram_tensor(f"cc_in{g}", [C, TCH], F16) for g in range(NTC)]
    cc_out = [nc.dram_tensor(f"cc_out{g}", [256, TCH], F16)
              for g in range(NTC)]
    groups = [[0, 1, 2, 3], [4, 5, 6, 7]]

    with tile.TileContext(nc) as tc:
        with (
            tc.tile_pool(name="spsum", bufs=2, space="PSUM") as spool,
            tc.tile_pool(name="ypsum", bufs=1, space="PSUM") as ypsum,
            tc.tile_pool(name="mm", bufs=2, space="PSUM") as mmpool,
            tc.tile_pool(name="const", bufs=1) as cpool,
            tc.tile_pool(name="xt", bufs=16) as xpool,
            tc.tile_pool(name="tmp", bufs=6) as tpool,
            tc.tile_pool(name="exp", bufs=4) as epool,
            tc.tile_pool(name="ysb", bufs=4) as ypool,
            tc.tile_pool(name="osb", bufs=4) as opool,
            tc.tile_pool(name="small", bufs=4) as smpool,
        ):
            # ---- persistent SBUF tiles --------------------------------
            # (x chunk-0 + w_qk loads go first on the sync queue; wv and
            # the rope tables on the vector queue; wp/tri/ident/ones on
            # the scalar queue -- parallel DMA rings, and nothing the
            # first matmuls need is queued behind cold constants.)
            wqk_sb = []
            for c8 in range(8):
                w = cpool.tile([128, 512], F16, tag=f"wqk{c8}", name=f"wqk{c8}")
                wqk_sb.append(w)
            wv_sb = []
            for c8 in range(8):
                w = cpool.tile([128, 256], F16, tag=f"wv{c8}", name=f"wv{c8}")
                wv_sb.append(w)
            wp_sb = []
            for k2 in range(2):
                w = cpool.tile([128, C], F16, tag=f"wp{k2}", name=f"wp{k2}")
                wp_sb.append(w)
            cost_sb = cpool.tile([128, T], F16, tag="cost", name="cost_sb")
            sint_sb = cpool.tile([128, T], F16, tag="sint", name="sint_sb")
            tri_sb = cpool.tile([128, 128], F16, tag="tri", name="tri_sb")
            ident_sb = cpool.tile([128, 128], F16, tag="ident", name="ident_sb")
            ones_sb = cpool.tile([128, 64], F16, tag="ones", name="ones_sb")
            rope_q = [cpool.tile([128, T], F16, tag=f"rq{p}", name=f"rq{p}")
                      for p in range(2)]
            rope_k = [cpool.tile([128, T], F16, tag=f"rk{p}", name=f"rk{p}")
                      for p in range(2)]
            vaug = cpool.tile([128, NBLK, 4, 65], F16, tag="vaug",
                              name="vaug")

            def load_consts_a():  # needed by the first q/k matmuls + rope
                for c8 in range(8):
                    nc.sync.dma_start(out=wqk_sb[c8][:, :],
                                      in_=w_qk[c8 * 128:(c8 + 1) * 128, :])
                nc.sync.dma_start(out=cost_sb[:, :], in_=cost_c[:, :])
                nc.sync.dma_start(out=sint_sb[:, :], in_=sint_c[:, :])

            def load_consts_b():  # needed by v / diag of att(0)
                for c8 in range(8):
                    nc.sync.dma_start(out=wv_sb[c8][:, :],
                                      in_=w_v[c8 * 128:(c8 + 1) * 128, :])
                nc.sync.dma_start(out=tri_sb[:, :], in_=tri_c[:, :])
                nc.sync.dma_start(out=ident_sb[:, :], in_=ident_c[:, :])
                nc.sync.dma_start(out=ones_sb[:, :], in_=ones_c[:, :])
                nc.vector.memset(vaug[:, :, :, 64], 1.0)

            def load_consts_c():  # needed from OP(0) on
                for k2 in range(2):
                    nc.sync.dma_start(
                        out=wp_sb[k2][:, :],
                        in_=w_p[k2 * 128:(k2 + 1) * 128, :])

            # ---- QKV pieces -------------------------------------------
            def qkv_pieces(c):
                t0 = c * TCH
                xts = []

                def load_x():
                    for c8 in range(8):
                        xt = xpool.tile([128, TCH], F16, tag="xt", name="xt")
                        nc.sync.dma_start(
                            out=xt[:, :],
                            in_=x_t[c8 * 128:(c8 + 1) * 128, t0:t0 + TCH])
                        xts.append(xt)

                def qk_piece(p, base, rope_dst):
                    ps = mmpool.tile([128, TCH], F32, tag="mm", name="qkps")
                    for c8 in range(8):
                        nc.tensor.matmul(
                            ps[:, :], wqk_sb[c8][:, base:base + 128],
                            xts[c8][:, :], start=(c8 == 0), stop=(c8 == 7))
                    m_sb = tpool.tile([128, TCH], F16, tag="msb", name="msb")
                    nc.vector.tensor_copy(m_sb[:, :], ps[:, :])
                    ms_sb = tpool.tile([128, TCH], F16, tag="mssb",
                                       name="mssb")
                    nc.sync.dma_start(out=ms_sb[0::2, :], in_=m_sb[1::2, :])
                    nc.sync.dma_start(out=ms_sb[1::2, :], in_=m_sb[0::2, :])
                    t1 = tpool.tile([128, TCH], F16, tag="t1", name="t1")
                    nc.vector.tensor_mul(t1[:, :], m_sb[:, :],
                                         cost_sb[:, t0:t0 + TCH])
                    t2 = tpool.tile([128, TCH], F16, tag="t2", name="t2")
                    nc.vector.tensor_mul(t2[:, :], ms_sb[:, :],
                                         sint_sb[:, t0:t0 + TCH])
                    nc.vector.tensor_add(rope_dst[:, t0:t0 + TCH],
                                         t1[:, :], t2[:, :])

                def v_piece(j):  # two tk blocks per piece
                    vps = mmpool.tile([128, 2, 4, 64], F32, tag="mm",
                                      name="vps")
                    for i in range(2):
                        blk = 2 * j + i
                        for c8 in range(8):
                            nc.tensor.matmul(
                                vps[:, i, :, :],
                                xts[c8][:, 128 * blk:128 * blk + 128],
                                wv_sb[c8][:, :],
                                start=(c8 == 0), stop=(c8 == 7))
                    tb = 4 * c + 2 * j
                    nc.vector.tensor_copy(vaug[:, tb:tb + 2, :, 0:64],
                                          vps[:, :, :, :])

                pieces = [load_x]
                for p in range(2):
                    pieces.append(lambda p=p: qk_piece(p, 128 * p, rope_q[p]))
                    pieces.append(
                        lambda p=p: qk_piece(p, 256 + 128 * p, rope_k[p]))
                pieces.append(lambda: v_piece(0))
                pieces.append(lambda: v_piece(1))
                return pieces

            # ---- attention --------------------------------------------
            y_tiles = [None, None]

            def att_groups(g):
                """Bundles: per pair, per-group scores+exp (lag-1 AV),
                then normalize."""
                t0 = g * TCH
                bundles = []
                for p in range(2):
                    yps = [None, None]

                    def kq(h, Tt, c0, p=p):
                        return (rope_k[p][64 * h:64 * h + 64,
                                          128 * Tt:128 * Tt + 128],
                                rope_q[p][64 * h:64 * h + 64,
                                          t0 + c0:t0 + TCH])

                    def tri_mm(s2, off, stop=True):
                        nc.tensor.matmul(s2[:, off:off + 128],
                                         ident_sb[:, :], tri_sb[:, :],
                                         start=False, stop=stop)

                    def nd_group(Tt, p=p, yps=yps, kq=kq):
                        # full tile: h0 @0:512 (bank0), h1 @512:1024
                        s2 = spool.tile([128, 2 * TCH], F32, tag="s",
                                        name="s2")
                        for h in range(2):
                            kk, qq = kq(h, Tt, 0)
                            nc.tensor.matmul(s2[:, 512 * h:512 * h + 512],
                                             kk, qq, start=True, stop=True)
                        e = epool.tile([128, 2 * TCH], F16, tag="e", name="e")
                        nc.scalar.activation(e[:, :], s2[:, :], EXP,
                                             scale=0.125)

                        def av(p=p, yps=yps, e=e, Tt=Tt):
                            for h in range(2):
                                nc.tensor.matmul(
                                    yps[h][:, :],
                                    vaug[:, Tt, 2 * p + h, :],
                                    e[:, 512 * h:512 * h + 512],
                                    start=(Tt == 0), stop=False)
                        return av

                    def r0_group(p=p, yps=yps, kq=kq, tri_mm=tri_mm):
                        Tt = 4 * g
                        s2 = spool.tile([128, 2 * TCH], F32, tag="s",
                                        name="s2r0")
                        for h in range(2):
                            kk, qq = kq(h, Tt, 0)
                            nc.tensor.matmul(s2[:, 512 * h:512 * h + 512],
                                             kk, qq, start=True, stop=False)
                        for h in range(2):
                            tri_mm(s2, 512 * h)
                        e = epool.tile([128, 2 * TCH], F16, tag="e",
                                       name="er0")
                        if debug and g == 0 and p == 0:
                            s32 = smpool.tile([128, 2 * TCH], F32, tag="s32",
                                              name="s32")
                            nc.vector.tensor_copy(s32[:, :], s2[:, :])
                            nc.sync.dma_start(out=dbg["s0"][:, :],
                                              in_=s32[:, :])
                        nc.scalar.activation(e[:, :], s2[:, :], EXP,
                                             scale=0.125)
                        if debug and g == 0 and p == 0:
                            nc.sync.dma_start(out=dbg["e0"][:, :], in_=e[:, :])

                        def av(p=p, yps=yps, e=e, Tt=Tt):
                            for h in range(2):
                                nc.tensor.matmul(
                                    yps[h][:, :],
                                    vaug[:, Tt, 2 * p + h, :],
                                    e[:, 512 * h:512 * h + 512],
                                    start=(g == 0), stop=False)
                        return av

                    def r1_group(p=p, yps=yps, kq=kq, tri_mm=tri_mm):
                        # h0 @0:384 (bank0), dead 384:512, h1 @512:896
                        Tt = 4 * g + 1
                        s2 = spool.tile([128, 2 * TCH], F32, tag="s",
                                        name="s2r1")
                        nc.vector.memset(s2[:, 384:512], NEG)
                        for h in range(2):
                            kk, qq = kq(h, Tt, 128)
                            nc.tensor.matmul(s2[:, 512 * h:512 * h + 384],
                                             kk, qq, start=True, stop=False)
                        for h in range(2):
                            tri_mm(s2, 512 * h)
                        e = epool.tile([128, 2 * TCH], F16, tag="e",
                                       name="er1")
                        nc.scalar.activation(e[:, 0:896], s2[:, 0:896], EXP,
                                             scale=0.125)

                        def av(p=p, yps=yps, e=e, Tt=Tt):
                            for h in range(2):
                                nc.tensor.matmul(
                                    yps[h][:, 128:TCH],
                                    vaug[:, Tt, 2 * p + h, :],
                                    e[:, 512 * h:512 * h + 384],
                                    start=False, stop=False)
                        return av

                    def d2_group(p=p, yps=yps, kq=kq, tri_mm=tri_mm):
                        # h0: r2 @0:256, r3 @256:384 (bank0); dead 384:512
                        # h1: r2 @512:768, r3 @768:896 (bank1)
                        s2 = spool.tile([128, 2 * TCH], F32, tag="s",
                                        name="s2d2")
                        nc.vector.memset(s2[:, 384:512], NEG)
                        for h in range(2):
                            kk, qq = kq(h, 4 * g + 2, 256)
                            nc.tensor.matmul(s2[:, 512 * h:512 * h + 256],
                                             kk, qq, start=True, stop=False)
                        for h in range(2):
                            kk, qq = kq(h, 4 * g + 3, 384)
                            nc.tensor.matmul(
                                s2[:, 512 * h + 256:512 * h + 384],
                                kk, qq, start=False, stop=False)
                        for h in range(2):
                            tri_mm(s2, 512 * h, stop=False)
                            tri_mm(s2, 512 * h + 256)
                        e = epool.tile([128, 2 * TCH], F16, tag="e",
                                       name="ed2")
                        nc.scalar.activation(e[:, 0:896], s2[:, 0:896], EXP,
                                             scale=0.125)

                        def av(p=p, yps=yps, e=e):
                            for h in range(2):
                                hb = 512 * h
                                nc.tensor.matmul(
                                    yps[h][:, 256:TCH],
                                    vaug[:, 4 * g + 2, 2 * p + h, :],
                                    e[:, hb:hb + 256],
                                    start=False, stop=False)
                                nc.tensor.matmul(
                                    yps[h][:, 384:TCH],
                                    vaug[:, 4 * g + 3, 2 * p + h, :],
                                    e[:, hb + 256:hb + 384],
                                    start=False, stop=True)
                        return av

                    def normalize(p=p, yps=yps):
                        y_sb = ypool.tile([128, TCH], F16, tag=f"ysb{p}",
                                          name=f"ysb{p}")
                        if debug and g == 0 and p == 0:
                            y32 = smpool.tile([65, TCH], F32, tag="y32",
                                              name="y32")
                            nc.vector.tensor_copy(y32[:, :], yps[0][:, :])
                            nc.sync.dma_start(out=dbg["yraw"][:, :],
                                              in_=y32[:, :])
                        for h in range(2):
                            den16 = smpool.tile([65, TCH], F16, tag="den16",
                                                name="den16")
                            nc.vector.tensor_copy(den16[64:65, :],
                                                  yps[h][64:65, :])
                            dbc = mmpool.tile([128, TCH], F32, tag="mm",
                                              name="dbc")
                            nc.tensor.matmul(dbc[0:64, :],
                                             ones_sb[64:65, :],
                                             den16[64:65, :], start=True,
                                             stop=True)
                            rec = smpool.tile([64, TCH], F32, tag="rec",
                                              name="rec")
                            nc.vector.reciprocal_approx_fast(
                                rec[:, :], dbc[0:64, :])
                            nc.vector.scalar_tensor_tensor(
                                y_sb[64 * h:64 * h + 64, :],
                                yps[h][0:64, :], 1.0, rec[:, :],
                                op0=MUL, op1=MUL)
                            if debug and g == 0 and p == 0 and h == 0:
                                nc.sync.dma_start(out=dbg["rec"][:, :],
                                                  in_=rec[0:1, :])
                        if debug and g == 0 and p == 0:
                            nc.sync.dma_start(out=dbg["ysb"][:, :],
                                              in_=y_sb[:, :])
                        y_tiles[p] = y_sb

                    state = {"av": None}
                    grp_fns = [lambda Tt=Tt, f=nd_group: f(Tt)
                               for Tt in range(4 * g)]
                    grp_fns += [r0_group, r1_group, d2_group]

                    def piped(spec, first=False, yps=yps, state=state):
                        def run():
                            if first:
                                for h in range(2):
                                    yps[h] = ypsum.tile(
                                        [65, TCH], F32, tag=f"y{h}",
                                        name=f"y{h}")
                            av = spec()
                            prev = state["av"]
                            state["av"] = av
                            if prev:
                                prev()
                        return run

                    def tail(fn=normalize, state=state):
                        def run():
                            state["av"]()
                            state["av"] = None
                            fn()
                        return run

                    bundles.append(piped(grp_fns[0], first=True))
                    bundles.extend(piped(f) for f in grp_fns[1:])
                    bundles.append(tail())
                return bundles

            # ---- output projection + collectives ----------------------
            def op_pieces(g):
                """8 m-tile pieces with a half ReduceScatter after each
                4; captures the y tiles at call time."""
                y0, y1 = y_tiles[0], y_tiles[1]
                pieces = []

                def m_piece(m8, y0=y0, y1=y1):
                    op_ps = mmpool.tile([128, TCH], F32, tag="mm",
                                        name="opps")
                    nc.tensor.matmul(op_ps[:, :],
                                     wp_sb[0][:, 128 * m8:128 * m8 + 128],
                                     y0[:, :], start=True, stop=False)
                    nc.tensor.matmul(op_ps[:, :],
                                     wp_sb[1][:, 128 * m8:128 * m8 + 128],
                                     y1[:, :], start=False, stop=True)
                    o_sb = opool.tile([128, TCH], F16, tag="osb", name="osb")
                    if m8 % 2 == 0:
                        nc.vector.tensor_copy(o_sb[:, :], op_ps[:, :])
                    else:
                        nc.scalar.copy(o_sb[:, :], op_ps[:, :])
                    # keep the sync queue free for x loads / rope swaps:
                    # collective-adjacent DMAs go on the scalar queue
                    nc.scalar.dma_start(
                        out=cc_in[g][128 * m8:128 * m8 + 128, :],
                        in_=o_sb[:, :])

                def rs():
                    nc.gpsimd.collective_compute(
                        "ReduceScatter", mybir.AluOpType.add,
                        replica_groups=groups,
                        ins=[cc_in[g].ap().opt()],
                        outs=[cc_out[g].ap().opt()])
                    nc.gpsimd.dma_start(out=out_ext[g, :, :],
                                        in_=cc_out[g][:, :])

                for m8 in range(8):
                    pieces.append(lambda m8=m8: m_piece(m8))
                pieces.append(rs)
                return pieces

            def interleave(bundles, fillers):
                nb, nf = len(bundles), len(fillers)
                fi = 0
                for i, bnd in enumerate(bundles):
                    bnd()
                    want = (i + 1) * nf // nb
                    while fi < want:
                        fillers[fi]()
                        fi += 1
                while fi < nf:
                    fillers[fi]()
                    fi += 1

            # ---- main emission ----------------------------------------
            load_consts_a()
            qkv0 = qkv_pieces(0)
            qkv0[0]()          # x chunk-0 loads right after w_qk
            load_consts_b()
            for f in qkv0[1:]:
                f()
            load_consts_c()
            pending_op = []
            for g in range(NTC):
                fillers = []
                if g + 1 < NTC:
                    fillers += qkv_pieces(g + 1)
                fillers += pending_op
                interleave(att_groups(g), fillers)
                pending_op = op_pieces(g)
            for f in pending_op:
                f()

    if not nc.is_finalized():
        nc.finalize()
    return nc


_NC_CACHE = None


def _get_nc():
    global _NC_CACHE
    if _NC_CACHE is None:
        _NC_CACHE = build()
    return _NC_CACHE


def make_in_maps(x, w_qkv, w_proj):
    x = np.asarray(x, np.float32)
    w_qkv = np.asarray(w_qkv, np.float32)
    w_proj = np.asarray(w_proj, np.float32)
    x_tb = [np.ascontiguousarray(x[b].T).astype(np.float16)
            for b in range(B)]
    maps = []
    for r in range(NCORE):
        b, q = divmod(r, 4)
        heads = [4 * q + i for i in range(4)]
        qcols, kcols = [], []
        for p in range(2):
            hs = heads[2 * p:2 * p + 2]
            rows = [h * 64 + d for h in hs for d in range(D)]
            qcols.append(w_qkv[rows, :].T)
            kcols.append(w_qkv[[C + i for i in rows], :].T)
        w_qk = np.ascontiguousarray(
            np.concatenate(qcols + kcols, axis=1)).astype(np.float16)
        vrows = [2 * C + h * 64 + d for h in heads for d in range(D)]
        w_v = np.ascontiguousarray(w_qkv[vrows, :].T).astype(np.float16)
        mydims = [h * 64 + d for h in heads for d in range(D)]
        w_p = np.ascontiguousarray(w_proj[:, mydims].T).astype(np.float16)
        maps.append({"x_t": x_tb[b], "w_qk": w_qk, "w_v": w_v, "w_p": w_p})
    return maps


def assemble(results):
    outT = np.zeros((B, C, T), np.float32)
    for r in range(NCORE):
        b, q = divmod(r, 4)
        o = results[r]["out"].astype(np.float32)  # [4, 256, TCH]
        for g in range(NTC):
            outT[b, 256 * q:256 * (q + 1), g * TCH:(g + 1) * TCH] = o[g]
    return np.ascontiguousarray(outT.transpose(0, 2, 1))


def run(x, w_qkv, w_proj, trace=False):
    nc = _get_nc()
    in_maps = make_in_maps(x, w_qkv, w_proj)
    res = run_bass_kernel_spmd(nc, in_maps, list(range(NCORE)), trace=trace)
    return assemble(res.results), res


def kernel(x, w_qkv, w_proj):
    out, _ = run(x, w_qkv, w_proj, trace=False)
    return out
